# revision 10
# baseline (speedup 1.0000x reference)
"""Trainium2 Bass kernel for nn_ConditionalFeedForward (MoE routed SwiGLU FFN).

Strategy (expert-parallel, routed):
  - Only the routed (token, expert) pairs are needed: on the host we bucket
    tokens by expert (deduplicating tokens that pick the same expert twice),
    pad each bucket to a fixed capacity C (= max bucket rounded up to 8),
    and give expert e's bucket to NeuronCore e (E=8 experts, 8 cores).
  - All operands are cast to bf16 on the host (free): halves DMA traffic,
    enables the compiler's fast-weight-load path, and keeps rel-err ~4e-3
    (threshold 2e-2).  Accumulation stays fp32 in PSUM.
  - Each core computes  yT = w2e @ (silu(w1e xg) * (w3e xg))  for its C
    routed tokens, everything SBUF-resident.
  - The host transposes/casts yT back and scatters rows into (T, TOP_K, D).

Device dataflow per core:
  warmup:   7 matmuls on a memset tile keep the PE HAM busy while the first
            DMAs land, so real matmuls run at the 2.4 GHz warm clock early.
  phase 1:  h1/h3 tiles [h=128, c-chunk<=512] accumulate over d in PSUM;
            Silu+mul drain into resident SBUF tensors a_lo/a_hi (bf16).
            The first h-tile's weights arrive via dedicated contiguous
            tensors (w1h0c/w3h0c) so the head DMAs run at line rate.
            w2 is prefetched into SBUF (two h-tiles per loop iteration,
            starting at h=1 to keep the head DMA window clear), so phase 2
            has no input DMA at all.
  phase 2:  yT[dd=128, c-chunk] accumulates over all 32 h-tiles in PSUM
            (lhsT = resident w2 tile [128,128] -> few, contiguous weight
            loads), in dd-groups of [2,2,2,1,1] so drains overlap the next
            group's matmuls and the final drain is small; the last drains
            split across the vector/scalar engines and both HWDGE rings.
"""

import numpy as np
import ml_dtypes

import sys

for _p in ("/opt/trn_rl_repo", "/root/.axon_site/_ro/trn_rl_repo"):
    if _p not in sys.path:
        sys.path.append(_p)

T = 4096
E = 8
D = 1024
H = 4096
TOP_K = 2
P = 128  # SBUF partitions

BF16 = ml_dtypes.bfloat16

_PROG_CACHE: dict = {}
LAST_RUN = None  # BassKernelResults of the most recent device run (for test.py)


def _chunks(C: int, width: int = 512):
    """Split [0, C) into (start, width) chunks each <= width."""
    out = []
    c0 = 0
    while c0 < C:
        w = min(width, C - c0)
        out.append((c0, w))
        c0 += w
    return out


def _build_program(C: int, Dm: int, Hm: int):
    """Emit the per-core Bass/Tile program for capacity-C routed tokens."""
    import concourse.bass as bass  # noqa: F401
    import concourse.mybir as mybir
    from concourse import bacc
    from concourse.tile import TileContext

    f32 = mybir.dt.float32
    bf16 = mybir.dt.bfloat16
    SILU = mybir.ActivationFunctionType.Silu
    COPY = mybir.ActivationFunctionType.Copy

    KD = Dm // P          # d-tiles (contraction of phase 1)
    NH = Hm // P          # h-tiles
    NHL = NH // 2         # h-tiles in a_lo
    CCH = _chunks(C)      # token chunks (<=512 wide)
    ND = Dm // P          # output-dim 128-tiles (phase 2)

    nc = bacc.Bacc("TRN2", target_bir_lowering=False)

    xgt = nc.dram_tensor("xgt", [Dm, C], bf16, kind="ExternalInput")
    w1t = nc.dram_tensor("w1t", [Dm, Hm], bf16, kind="ExternalInput")
    w3t = nc.dram_tensor("w3t", [Dm, Hm], bf16, kind="ExternalInput")
    w2t = nc.dram_tensor("w2t", [Hm, Dm], bf16, kind="ExternalInput")
    # first h-tile of w1/w3 pre-tiled on the host: [di, do*128+hh], contiguous
    w1h0c = nc.dram_tensor("w1h0c", [P, KD * P], bf16, kind="ExternalInput")
    w3h0c = nc.dram_tensor("w3h0c", [P, KD * P], bf16, kind="ExternalInput")
    yt = nc.dram_tensor("yt", [Dm, C], bf16, kind="ExternalOutput")

    xgt_r = xgt[:].rearrange("(do di) c -> di do c", di=P)   # [128, KD, C]
    w1t_r = w1t[:].rearrange("(do di) h -> di do h", di=P)   # [128, KD, H]
    w3t_r = w3t[:].rearrange("(do di) h -> di do h", di=P)

    with TileContext(nc) as tc:
        with (
            tc.tile_pool(name="xg", bufs=1) as xg_pool,
            tc.tile_pool(name="abuf", bufs=1) as a_pool,
            tc.tile_pool(name="w2r", bufs=1) as w2_pool,
            tc.tile_pool(name="w13", bufs=2) as w13_pool,
            tc.tile_pool(name="scratch", bufs=4) as scratch_pool,
            tc.tile_pool(name="ydrain", bufs=4) as y_pool,
            tc.tile_pool(name="warm", bufs=1) as warm_pool,
        ):
            # ---- HAM warm-up: PE busy from program start (no DMA deps) ----
            wu_sb = warm_pool.tile([P, 512], bf16)
            nc.gpsimd.memset(wu_sb, 0.0)

            # ---- head DMAs, priority order, spread over both HWDGE rings ----
            # sync ring: weights; scalar ring: xg (w2 prefetch on gpsimd).
            # The first matmul's deps are split out tiny (w1 d0 slice + xg
            # d0/cc0 chunk) so it can fire ~2us earlier.
            w1h0c_r = w1h0c[:].rearrange("p (do h) -> p do h", do=KD)
            w3h0c_r = w3h0c[:].rearrange("p (do h) -> p do h", do=KD)
            w1_sb0 = w13_pool.tile([P, KD, P], bf16, tag="w1h0")
            nc.sync.dma_start(out=w1_sb0[:, 0:1, :], in_=w1h0c_r[:, 0:1, :])
            xg_sb = xg_pool.tile([P, KD, C], bf16)
            c00, cw0 = CCH[0]
            nc.scalar.dma_start(
                out=xg_sb[:, 0, c00 : c00 + cw0], in_=xgt_r[:, 0, c00 : c00 + cw0]
            )
            nc.sync.dma_start(out=w1_sb0[:, 1:KD, :], in_=w1h0c_r[:, 1:KD, :])
            for c0, cw in CCH[1:]:
                nc.scalar.dma_start(
                    out=xg_sb[:, 0, c0 : c0 + cw], in_=xgt_r[:, 0, c0 : c0 + cw]
                )
            w3_sb0 = w13_pool.tile([P, KD, P], bf16, tag="w3h0")
            nc.sync.dma_start(out=w3_sb0, in_=w3h0c_r)
            for d in range(1, KD):
                nc.scalar.dma_start(out=xg_sb[:, d, :], in_=xgt_r[:, d, :])

            a_lo = a_pool.tile([P, NHL, C], bf16, tag="alo")
            a_hi = a_pool.tile([P, NH - NHL, C], bf16, tag="ahi")

            def a_slice(h, c0, cw):
                if h < NHL:
                    return a_lo[:, h, c0 : c0 + cw]
                return a_hi[:, h - NHL, c0 : c0 + cw]

            w2_sb = w2_pool.tile([P, NH, Dm], bf16)

            # ---- one PSUM pool for warmup + both phases: no pool-close
            # ---- barrier between phase 1's last drains and phase 2
            with tc.tile_pool(name="ps", bufs=8, space="PSUM") as ps1:
                ps2 = ps1
                wu_ps = ps1.tile([P, 512], f32, tag="ps", name="warm")
                for _ in range(2):
                    nc.tensor.matmul(wu_ps, wu_sb[:, 0:P], wu_sb, start=True, stop=True)
                # preload the Silu activation table on the scalar engine now
                # (first real use would otherwise stall drains ~1.3us mid-head)
                silu_warm = scratch_pool.tile([P, 8], f32, tag="actwarm")
                nc.scalar.activation(silu_warm, wu_sb[:, 0:8], SILU)

                # ---- phase 1: a[h, c] = silu(w1.T x) * (w3.T x) ----
                for h in range(NH):
                    hs = slice(h * P, (h + 1) * P)
                    if h == 0:
                        w1_sb, w3_sb = w1_sb0, w3_sb0
                    else:
                        w1_sb = w13_pool.tile([P, KD, P], bf16, tag="w1")
                        nc.sync.dma_start(out=w1_sb, in_=w1t_r[:, :, hs])
                        w3_sb = w13_pool.tile([P, KD, P], bf16, tag="w3")
                        nc.sync.dma_start(out=w3_sb, in_=w3t_r[:, :, hs])
                        # w2 prefetch for phase 2 (gpsimd SWDGE ring: keeps
                        # the scalar queue free for the silu drains)
                        for h2 in (2 * (h - 1), 2 * (h - 1) + 1):
                            if h2 < NH:
                                nc.gpsimd.dma_start(
                                    out=w2_sb[:, h2, :],
                                    in_=w2t[h2 * P : (h2 + 1) * P, :],
                                )

                    h1_ps = [
                        ps1.tile([P, cw], f32, tag="ps", name=f"h1_{h}_{i}")
                        for i, (c0, cw) in enumerate(CCH)
                    ]
                    h3_ps = [
                        ps1.tile([P, cw], f32, tag="ps", name=f"h3_{h}_{i}")
                        for i, (c0, cw) in enumerate(CCH)
                    ]
                    # h==0 is paced by xg arrival: interleave w1/w3 per
                    # d-tile so each arriving xg slice unlocks 4 matmuls.
                    if h == 0:
                        wave = [(w1_sb, h1_ps), (w3_sb, h3_ps)]
                        for d in range(KD):
                            for w_sb, ps in wave:
                                for i, (c0, cw) in enumerate(CCH):
                                    nc.tensor.matmul(
                                        ps[i],
                                        w_sb[:, d, :],
                                        xg_sb[:, d, c0 : c0 + cw],
                                        start=(d == 0),
                                        stop=(d == KD - 1),
                                    )
                    else:
                        for d in range(KD):
                            for i, (c0, cw) in enumerate(CCH):
                                nc.tensor.matmul(
                                    h1_ps[i],
                                    w1_sb[:, d, :],
                                    xg_sb[:, d, c0 : c0 + cw],
                                    start=(d == 0),
                                    stop=(d == KD - 1),
                                )
                        for d in range(KD):
                            for i, (c0, cw) in enumerate(CCH):
                                nc.tensor.matmul(
                                    h3_ps[i],
                                    w3_sb[:, d, :],
                                    xg_sb[:, d, c0 : c0 + cw],
                                    start=(d == 0),
                                    stop=(d == KD - 1),
                                )
                    for i, (c0, cw) in enumerate(CCH):
                        s_sb = scratch_pool.tile([P, 512], f32, tag="scratch")
                        nc.scalar.activation(s_sb[:, 0:cw], h1_ps[i], SILU)
                        nc.vector.tensor_mul(
                            out=a_slice(h, c0, cw),
                            in0=s_sb[:, 0:cw],
                            in1=h3_ps[i],
                        )

                # ---- phase 2: yT[dd, c] = sum_h w2T[h, dd].T @ a[h, c] ----
                # dd-groups sized [2,2,2,1,1]: drains overlap the next
                # group's matmuls and the final drain is small.
                ddgs = []
                nd = 0
                for g in (2, 2, 2, 1, 1):
                    ddgs.append(list(range(nd, min(nd + g, ND))))
                    nd += g
                assert nd == ND, (nd, ND)

                for gi, dds in enumerate(ddgs):
                    y_ps = {
                        (dd, i): ps2.tile([P, cw], f32, tag="ps", name=f"y_{dd}_{i}")
                        for dd in dds
                        for i, (c0, cw) in enumerate(CCH)
                    }
                    for h in range(NH):
                        for dd in dds:
                            w2_w = w2_sb[:, h, dd * P : (dd + 1) * P]
                            for i, (c0, cw) in enumerate(CCH):
                                nc.tensor.matmul(
                                    y_ps[(dd, i)],
                                    w2_w,
                                    a_slice(h, c0, cw),
                                    start=(h == 0),
                                    stop=(h == NH - 1),
                                )
                    last = gi == len(ddgs) - 1
                    for dd in dds:
                        for i, (c0, cw) in enumerate(CCH):
                            y_sb = y_pool.tile(
                                [P, 512], bf16, tag="y", name=f"ysb_{dd}_{i}"
                            )
                            if last and i % 2 == 1:
                                # parallelize the final drain across engines
                                nc.scalar.activation(
                                    y_sb[:, 0:cw], y_ps[(dd, i)], COPY
                                )
                                nc.scalar.dma_start(
                                    out=yt[dd * P : (dd + 1) * P, c0 : c0 + cw],
                                    in_=y_sb[:, 0:cw],
                                )
                            else:
                                nc.vector.tensor_copy(
                                    out=y_sb[:, 0:cw], in_=y_ps[(dd, i)]
                                )
                                nc.sync.dma_start(
                                    out=yt[dd * P : (dd + 1) * P, c0 : c0 + cw],
                                    in_=y_sb[:, 0:cw],
                                )
    nc.compile()  # bacc passes: split multi-waits, alloc regs, fuse nops
    return nc


def _get_program(C: int, Dm: int, Hm: int):
    key = (C, Dm, Hm)
    if key not in _PROG_CACHE:
        _PROG_CACHE[key] = _build_program(C, Dm, Hm)
    return _PROG_CACHE[key]


def kernel(x, expert_indices, w1, w2, w3):
    global LAST_RUN
    from concourse.bass_utils import run_bass_kernel_spmd

    x = np.ascontiguousarray(np.asarray(x, dtype=np.float32))
    idx = np.asarray(expert_indices)
    w1 = np.asarray(w1, dtype=np.float32)
    w2 = np.asarray(w2, dtype=np.float32)
    w3 = np.asarray(w3, dtype=np.float32)

    Tn, Kn = idx.shape
    Dm = x.shape[1]
    En, Hm, _ = w1.shape
    assert En == 8, f"kernel is hardcoded for 8 experts on 8 cores, got {En}"
    idx64 = idx.astype(np.int64)
    KD = Dm // P

    # Host routing: unique token list per expert.
    toks = [np.nonzero((idx64 == e).any(axis=1))[0] for e in range(En)]
    maxc = max(len(t) for t in toks)
    C = max(512, -(-maxc // 8) * 8)

    nc = _get_program(C, Dm, Hm)

    in_maps = []
    for e in range(En):
        te = toks[e]
        xg = np.zeros((C, Dm), np.float32)
        xg[: len(te)] = x[te]
        w1te = np.ascontiguousarray(w1[e].T.astype(BF16))  # [Dm, Hm]
        w3te = np.ascontiguousarray(w3[e].T.astype(BF16))
        # first h-tile pre-tiled: [di, do, hh] contiguous per partition
        w1h0c = np.ascontiguousarray(
            w1te[:, 0:P].reshape(KD, P, P).transpose(1, 0, 2).reshape(P, KD * P)
        )
        w3h0c = np.ascontiguousarray(
            w3te[:, 0:P].reshape(KD, P, P).transpose(1, 0, 2).reshape(P, KD * P)
        )
        in_maps.append(
            {
                "xgt": np.ascontiguousarray(xg.T.astype(BF16)),
                "w1t": w1te,
                "w3t": w3te,
                "w2t": np.ascontiguousarray(w2[e].T.astype(BF16)),
                "w1h0c": w1h0c,
                "w3h0c": w3h0c,
            }
        )

    LAST_RUN = run_bass_kernel_spmd(nc, in_maps, list(range(En)))
    res = LAST_RUN.results

    out = np.empty((Tn, Kn, Dm), np.float32)
    for e in range(En):
        ye = np.asarray(res[e]["yt"]).astype(np.float32).T  # [C, Dm]
        t_arr, k_arr = np.nonzero(idx64 == e)
        pos = np.searchsorted(toks[e], t_arr)
        out[t_arr, k_arr] = ye[pos]
    return out


# revision 11
# speedup vs baseline: 1.0437x; 1.0437x over previous
"""Trainium2 Bass kernel for nn_ConditionalFeedForward (MoE routed SwiGLU FFN).

Strategy (expert-parallel, routed):
  - Only the routed (token, expert) pairs are needed: on the host we bucket
    tokens by expert (deduplicating tokens that pick the same expert twice),
    pad each bucket to a fixed capacity C (= max bucket rounded up to 8),
    and give expert e's bucket to NeuronCore e (E=8 experts, 8 cores).
  - All operands are cast to bf16 on the host (free): halves DMA traffic,
    enables the compiler's fast-weight-load path, and keeps rel-err ~4e-3
    (threshold 2e-2).  Accumulation stays fp32 in PSUM.
  - Each core computes  yT = w2e @ (silu(w1e xg) * (w3e xg))  for its C
    routed tokens, everything SBUF-resident.
  - The host transposes/casts yT back and scatters rows into (T, TOP_K, D).

Device dataflow per core:
  warmup:   7 matmuls on a memset tile keep the PE HAM busy while the first
            DMAs land, so real matmuls run at the 2.4 GHz warm clock early.
  phase 1:  h1/h3 tiles [h=128, c-chunk<=512] accumulate over d in PSUM;
            Silu+mul drain into resident SBUF tensors a_lo/a_hi (bf16).
            The first h-tile's weights arrive via dedicated contiguous
            tensors (w1h0c/w3h0c) so the head DMAs run at line rate.
            w2 is prefetched into SBUF (two h-tiles per loop iteration,
            starting at h=1 to keep the head DMA window clear), so phase 2
            has no input DMA at all.
  phase 2:  yT[dd=128, c-chunk] accumulates over all 32 h-tiles in PSUM
            (lhsT = resident w2 tile [128,128] -> few, contiguous weight
            loads), in dd-groups of [2,2,2,1,1] so drains overlap the next
            group's matmuls and the final drain is small; the last drains
            split across the vector/scalar engines and both HWDGE rings.
"""

import numpy as np
import ml_dtypes

import sys

for _p in ("/opt/trn_rl_repo", "/root/.axon_site/_ro/trn_rl_repo"):
    if _p not in sys.path:
        sys.path.append(_p)

T = 4096
E = 8
D = 1024
H = 4096
TOP_K = 2
P = 128  # SBUF partitions

BF16 = ml_dtypes.bfloat16

_PROG_CACHE: dict = {}
LAST_RUN = None  # BassKernelResults of the most recent device run (for test.py)


def _chunks(C: int, width: int = 512):
    """Split [0, C) into (start, width) chunks each <= width."""
    out = []
    c0 = 0
    while c0 < C:
        w = min(width, C - c0)
        out.append((c0, w))
        c0 += w
    return out


def _build_program(C: int, Dm: int, Hm: int):
    """Emit the per-core Bass/Tile program for capacity-C routed tokens."""
    import concourse.bass as bass  # noqa: F401
    import concourse.mybir as mybir
    from concourse import bacc
    from concourse.tile import TileContext

    f32 = mybir.dt.float32
    bf16 = mybir.dt.bfloat16
    SILU = mybir.ActivationFunctionType.Silu
    COPY = mybir.ActivationFunctionType.Copy

    KD = Dm // P          # d-tiles (contraction of phase 1)
    NH = Hm // P          # h-tiles
    NHL = NH // 2         # h-tiles in a_lo
    CCH = _chunks(C)      # token chunks (<=512 wide)
    ND = Dm // P          # output-dim 128-tiles (phase 2)

    nc = bacc.Bacc("TRN2", target_bir_lowering=False)

    xgt = nc.dram_tensor("xgt", [Dm, C], bf16, kind="ExternalInput")
    w1t = nc.dram_tensor("w1t", [Dm, Hm], bf16, kind="ExternalInput")
    w3t = nc.dram_tensor("w3t", [Dm, Hm], bf16, kind="ExternalInput")
    w2t = nc.dram_tensor("w2t", [Hm, Dm], bf16, kind="ExternalInput")
    # first h-tile of w1/w3 pre-tiled on the host: [di, do*128+hh], contiguous
    w1h0c = nc.dram_tensor("w1h0c", [P, KD * P], bf16, kind="ExternalInput")
    w3h0c = nc.dram_tensor("w3h0c", [P, KD * P], bf16, kind="ExternalInput")
    yt = nc.dram_tensor("yt", [Dm, C], bf16, kind="ExternalOutput")

    xgt_r = xgt[:].rearrange("(do di) c -> di do c", di=P)   # [128, KD, C]
    w1t_r = w1t[:].rearrange("(do di) h -> di do h", di=P)   # [128, KD, H]
    w3t_r = w3t[:].rearrange("(do di) h -> di do h", di=P)

    with TileContext(nc) as tc:
        with (
            tc.tile_pool(name="xg", bufs=1) as xg_pool,
            tc.tile_pool(name="abuf", bufs=1) as a_pool,
            tc.tile_pool(name="w2r", bufs=1) as w2_pool,
            tc.tile_pool(name="w13", bufs=2) as w13_pool,
            tc.tile_pool(name="scratch", bufs=4) as scratch_pool,
            tc.tile_pool(name="ydrain", bufs=4) as y_pool,
            tc.tile_pool(name="warm", bufs=1) as warm_pool,
        ):
            # ---- HAM warm-up: PE busy from program start (no DMA deps) ----
            wu_sb = warm_pool.tile([P, 512], bf16)
            nc.gpsimd.memset(wu_sb, 0.0)

            # ---- head DMAs, priority order, spread over both HWDGE rings ----
            # sync ring: weights; scalar ring: xg (w2 prefetch on gpsimd).
            # The first matmul's deps are split out tiny (w1 d0 slice + xg
            # d0/cc0 chunk) so it can fire ~2us earlier.
            w1h0c_r = w1h0c[:].rearrange("p (do h) -> p do h", do=KD)
            w3h0c_r = w3h0c[:].rearrange("p (do h) -> p do h", do=KD)
            w1_sb0 = w13_pool.tile([P, KD, P], bf16, tag="w1h0")
            nc.sync.dma_start(out=w1_sb0[:, 0:1, :], in_=w1h0c_r[:, 0:1, :])
            xg_sb = xg_pool.tile([P, KD, C], bf16)
            c00, cw0 = CCH[0]
            nc.scalar.dma_start(
                out=xg_sb[:, 0, c00 : c00 + cw0], in_=xgt_r[:, 0, c00 : c00 + cw0]
            )
            nc.sync.dma_start(out=w1_sb0[:, 1:KD, :], in_=w1h0c_r[:, 1:KD, :])
            for c0, cw in CCH[1:]:
                nc.scalar.dma_start(
                    out=xg_sb[:, 0, c0 : c0 + cw], in_=xgt_r[:, 0, c0 : c0 + cw]
                )
            w3_sb0 = w13_pool.tile([P, KD, P], bf16, tag="w3h0")
            nc.sync.dma_start(out=w3_sb0, in_=w3h0c_r)
            for d in range(1, KD):
                nc.scalar.dma_start(out=xg_sb[:, d, :], in_=xgt_r[:, d, :])

            a_lo = a_pool.tile([P, NHL, C], bf16, tag="alo")
            a_hi = a_pool.tile([P, NH - NHL, C], bf16, tag="ahi")

            def a_slice(h, c0, cw):
                if h < NHL:
                    return a_lo[:, h, c0 : c0 + cw]
                return a_hi[:, h - NHL, c0 : c0 + cw]

            w2_sb = w2_pool.tile([P, NH, Dm], bf16)

            # ---- one PSUM pool for warmup + both phases: no pool-close
            # ---- barrier between phase 1's last drains and phase 2
            with tc.tile_pool(name="ps", bufs=8, space="PSUM") as ps1:
                ps2 = ps1
                wu_ps = ps1.tile([P, 512], f32, tag="ps", name="warm")
                for _ in range(2):
                    nc.tensor.matmul(wu_ps, wu_sb[:, 0:P], wu_sb, start=True, stop=True)
                # preload the Silu activation table on the scalar engine now
                # (first real use would otherwise stall drains ~1.3us mid-head)
                silu_warm = scratch_pool.tile([P, 8], f32, tag="actwarm")
                nc.scalar.activation(silu_warm, wu_sb[:, 0:8], SILU)

                # ---- phase 1: a[h, c] = silu(w1.T x) * (w3.T x) ----
                for h in range(NH):
                    hs = slice(h * P, (h + 1) * P)
                    if h == 0:
                        w1_sb, w3_sb = w1_sb0, w3_sb0
                    else:
                        w1_sb = w13_pool.tile([P, KD, P], bf16, tag="w1")
                        nc.sync.dma_start(out=w1_sb, in_=w1t_r[:, :, hs])
                        w3_sb = w13_pool.tile([P, KD, P], bf16, tag="w3")
                        nc.sync.dma_start(out=w3_sb, in_=w3t_r[:, :, hs])
                        # w2 prefetch for phase 2: scalar ring, starting only
                        # after the head xg/w13 window (the first ~25us of
                        # DMA is saturated with critical-path traffic)
                        if h >= 6:
                            for h2 in (2 * (h - 6), 2 * (h - 6) + 1):
                                if h2 < NH:
                                    nc.scalar.dma_start(
                                        out=w2_sb[:, h2, :],
                                        in_=w2t[h2 * P : (h2 + 1) * P, :],
                                    )

                    h1_ps = [
                        ps1.tile([P, cw], f32, tag="ps", name=f"h1_{h}_{i}")
                        for i, (c0, cw) in enumerate(CCH)
                    ]
                    h3_ps = [
                        ps1.tile([P, cw], f32, tag="ps", name=f"h3_{h}_{i}")
                        for i, (c0, cw) in enumerate(CCH)
                    ]
                    # h==0 is paced by xg arrival: interleave w1/w3 per
                    # d-tile so each arriving xg slice unlocks 4 matmuls.
                    if h == 0:
                        wave = [(w1_sb, h1_ps), (w3_sb, h3_ps)]
                        for d in range(KD):
                            for w_sb, ps in wave:
                                for i, (c0, cw) in enumerate(CCH):
                                    nc.tensor.matmul(
                                        ps[i],
                                        w_sb[:, d, :],
                                        xg_sb[:, d, c0 : c0 + cw],
                                        start=(d == 0),
                                        stop=(d == KD - 1),
                                    )
                    else:
                        for d in range(KD):
                            for i, (c0, cw) in enumerate(CCH):
                                nc.tensor.matmul(
                                    h1_ps[i],
                                    w1_sb[:, d, :],
                                    xg_sb[:, d, c0 : c0 + cw],
                                    start=(d == 0),
                                    stop=(d == KD - 1),
                                )
                        for d in range(KD):
                            for i, (c0, cw) in enumerate(CCH):
                                nc.tensor.matmul(
                                    h3_ps[i],
                                    w3_sb[:, d, :],
                                    xg_sb[:, d, c0 : c0 + cw],
                                    start=(d == 0),
                                    stop=(d == KD - 1),
                                )
                    for i, (c0, cw) in enumerate(CCH):
                        s_sb = scratch_pool.tile([P, 512], f32, tag="scratch")
                        nc.scalar.activation(s_sb[:, 0:cw], h1_ps[i], SILU)
                        nc.vector.tensor_mul(
                            out=a_slice(h, c0, cw),
                            in0=s_sb[:, 0:cw],
                            in1=h3_ps[i],
                        )

                # ---- phase 2: yT[dd, c] = sum_h w2T[h, dd].T @ a[h, c] ----
                # dd-groups sized [2,2,2,1,1]: drains overlap the next
                # group's matmuls and the final drain is small.
                ddgs = []
                nd = 0
                for g in (2, 2, 2, 1, 1):
                    ddgs.append(list(range(nd, min(nd + g, ND))))
                    nd += g
                assert nd == ND, (nd, ND)

                for gi, dds in enumerate(ddgs):
                    y_ps = {
                        (dd, i): ps2.tile([P, cw], f32, tag="ps", name=f"y_{dd}_{i}")
                        for dd in dds
                        for i, (c0, cw) in enumerate(CCH)
                    }
                    for h in range(NH):
                        for dd in dds:
                            w2_w = w2_sb[:, h, dd * P : (dd + 1) * P]
                            for i, (c0, cw) in enumerate(CCH):
                                nc.tensor.matmul(
                                    y_ps[(dd, i)],
                                    w2_w,
                                    a_slice(h, c0, cw),
                                    start=(h == 0),
                                    stop=(h == NH - 1),
                                )
                    last = gi == len(ddgs) - 1
                    for dd in dds:
                        for i, (c0, cw) in enumerate(CCH):
                            y_sb = y_pool.tile(
                                [P, 512], bf16, tag="y", name=f"ysb_{dd}_{i}"
                            )
                            if last and i % 2 == 1:
                                # parallelize the final drain across engines
                                nc.scalar.activation(
                                    y_sb[:, 0:cw], y_ps[(dd, i)], COPY
                                )
                                nc.scalar.dma_start(
                                    out=yt[dd * P : (dd + 1) * P, c0 : c0 + cw],
                                    in_=y_sb[:, 0:cw],
                                )
                            else:
                                nc.vector.tensor_copy(
                                    out=y_sb[:, 0:cw], in_=y_ps[(dd, i)]
                                )
                                nc.sync.dma_start(
                                    out=yt[dd * P : (dd + 1) * P, c0 : c0 + cw],
                                    in_=y_sb[:, 0:cw],
                                )
    nc.compile()  # bacc passes: split multi-waits, alloc regs, fuse nops
    return nc


def _get_program(C: int, Dm: int, Hm: int):
    key = (C, Dm, Hm)
    if key not in _PROG_CACHE:
        _PROG_CACHE[key] = _build_program(C, Dm, Hm)
    return _PROG_CACHE[key]


def kernel(x, expert_indices, w1, w2, w3):
    global LAST_RUN
    from concourse.bass_utils import run_bass_kernel_spmd

    x = np.ascontiguousarray(np.asarray(x, dtype=np.float32))
    idx = np.asarray(expert_indices)
    w1 = np.asarray(w1, dtype=np.float32)
    w2 = np.asarray(w2, dtype=np.float32)
    w3 = np.asarray(w3, dtype=np.float32)

    Tn, Kn = idx.shape
    Dm = x.shape[1]
    En, Hm, _ = w1.shape
    assert En == 8, f"kernel is hardcoded for 8 experts on 8 cores, got {En}"
    idx64 = idx.astype(np.int64)
    KD = Dm // P

    # Host routing: unique token list per expert.
    toks = [np.nonzero((idx64 == e).any(axis=1))[0] for e in range(En)]
    maxc = max(len(t) for t in toks)
    C = max(512, -(-maxc // 8) * 8)

    nc = _get_program(C, Dm, Hm)

    in_maps = []
    for e in range(En):
        te = toks[e]
        xg = np.zeros((C, Dm), np.float32)
        xg[: len(te)] = x[te]
        w1te = np.ascontiguousarray(w1[e].T.astype(BF16))  # [Dm, Hm]
        w3te = np.ascontiguousarray(w3[e].T.astype(BF16))
        # first h-tile pre-tiled: [di, do, hh] contiguous per partition
        w1h0c = np.ascontiguousarray(
            w1te[:, 0:P].reshape(KD, P, P).transpose(1, 0, 2).reshape(P, KD * P)
        )
        w3h0c = np.ascontiguousarray(
            w3te[:, 0:P].reshape(KD, P, P).transpose(1, 0, 2).reshape(P, KD * P)
        )
        in_maps.append(
            {
                "xgt": np.ascontiguousarray(xg.T.astype(BF16)),
                "w1t": w1te,
                "w3t": w3te,
                "w2t": np.ascontiguousarray(w2[e].T.astype(BF16)),
                "w1h0c": w1h0c,
                "w3h0c": w3h0c,
            }
        )

    LAST_RUN = run_bass_kernel_spmd(nc, in_maps, list(range(En)))
    res = LAST_RUN.results

    out = np.empty((Tn, Kn, Dm), np.float32)
    for e in range(En):
        ye = np.asarray(res[e]["yt"]).astype(np.float32).T  # [C, Dm]
        t_arr, k_arr = np.nonzero(idx64 == e)
        pos = np.searchsorted(toks[e], t_arr)
        out[t_arr, k_arr] = ye[pos]
    return out


# revision 13
# speedup vs baseline: 1.0453x; 1.0015x over previous
"""Trainium2 Bass kernel for nn_ConditionalFeedForward (MoE routed SwiGLU FFN).

Strategy (expert-parallel, routed):
  - Only the routed (token, expert) pairs are needed: on the host we bucket
    tokens by expert (deduplicating tokens that pick the same expert twice),
    pad each bucket to a fixed capacity C (= max bucket rounded up to 8),
    and give expert e's bucket to NeuronCore e (E=8 experts, 8 cores).
  - All operands are cast to bf16 on the host (free): halves DMA traffic,
    enables the compiler's fast-weight-load path, and keeps rel-err ~4e-3
    (threshold 2e-2).  Accumulation stays fp32 in PSUM.
  - Each core computes  yT = w2e @ (silu(w1e xg) * (w3e xg))  for its C
    routed tokens, everything SBUF-resident.
  - The host transposes/casts yT back and scatters rows into (T, TOP_K, D).

Device dataflow per core:
  warmup:   7 matmuls on a memset tile keep the PE HAM busy while the first
            DMAs land, so real matmuls run at the 2.4 GHz warm clock early.
  phase 1:  h1/h3 tiles [h=128, c-chunk<=512] accumulate over d in PSUM;
            Silu+mul drain into resident SBUF tensors a_lo/a_hi (bf16).
            The first h-tile's weights arrive via dedicated contiguous
            tensors (w1h0c/w3h0c) so the head DMAs run at line rate.
            w2 is prefetched into SBUF (two h-tiles per loop iteration,
            starting at h=1 to keep the head DMA window clear), so phase 2
            has no input DMA at all.
  phase 2:  yT[dd=128, c-chunk] accumulates over all 32 h-tiles in PSUM
            (lhsT = resident w2 tile [128,128] -> few, contiguous weight
            loads), in dd-groups of [2,2,2,1,1] so drains overlap the next
            group's matmuls and the final drain is small; the last drains
            split across the vector/scalar engines and both HWDGE rings.
"""

import numpy as np
import ml_dtypes

import sys

for _p in ("/opt/trn_rl_repo", "/root/.axon_site/_ro/trn_rl_repo"):
    if _p not in sys.path:
        sys.path.append(_p)

T = 4096
E = 8
D = 1024
H = 4096
TOP_K = 2
P = 128  # SBUF partitions

BF16 = ml_dtypes.bfloat16

_PROG_CACHE: dict = {}
LAST_RUN = None  # BassKernelResults of the most recent device run (for test.py)


def _chunks(C: int, width: int = 512):
    """Split [0, C) into (start, width) chunks each <= width."""
    out = []
    c0 = 0
    while c0 < C:
        w = min(width, C - c0)
        out.append((c0, w))
        c0 += w
    return out


def _build_program(C: int, Dm: int, Hm: int):
    """Emit the per-core Bass/Tile program for capacity-C routed tokens."""
    import concourse.bass as bass  # noqa: F401
    import concourse.mybir as mybir
    from concourse import bacc
    from concourse.tile import TileContext

    f32 = mybir.dt.float32
    bf16 = mybir.dt.bfloat16
    SILU = mybir.ActivationFunctionType.Silu
    COPY = mybir.ActivationFunctionType.Copy

    KD = Dm // P          # d-tiles (contraction of phase 1)
    NH = Hm // P          # h-tiles
    NHL = NH // 2         # h-tiles in a_lo
    CCH = _chunks(C)      # token chunks (<=512 wide)
    ND = Dm // P          # output-dim 128-tiles (phase 2)

    nc = bacc.Bacc("TRN2", target_bir_lowering=False)

    xgt = nc.dram_tensor("xgt", [Dm, C], bf16, kind="ExternalInput")
    w1t = nc.dram_tensor("w1t", [Dm, Hm], bf16, kind="ExternalInput")
    w3t = nc.dram_tensor("w3t", [Dm, Hm], bf16, kind="ExternalInput")
    w2t = nc.dram_tensor("w2t", [Hm, Dm], bf16, kind="ExternalInput")
    # first h-tile of w1/w3 pre-tiled on the host: [di, do*128+hh], contiguous
    w1h0c = nc.dram_tensor("w1h0c", [P, KD * P], bf16, kind="ExternalInput")
    w3h0c = nc.dram_tensor("w3h0c", [P, KD * P], bf16, kind="ExternalInput")
    yt = nc.dram_tensor("yt", [Dm, C], bf16, kind="ExternalOutput")

    xgt_r = xgt[:].rearrange("(do di) c -> di do c", di=P)   # [128, KD, C]
    w1t_r = w1t[:].rearrange("(do di) h -> di do h", di=P)   # [128, KD, H]
    w3t_r = w3t[:].rearrange("(do di) h -> di do h", di=P)

    with TileContext(nc) as tc:
        with (
            tc.tile_pool(name="xg", bufs=1) as xg_pool,
            tc.tile_pool(name="abuf", bufs=1) as a_pool,
            tc.tile_pool(name="w2r", bufs=1) as w2_pool,
            tc.tile_pool(name="w13", bufs=2) as w13_pool,
            tc.tile_pool(name="scratch", bufs=4) as scratch_pool,
            tc.tile_pool(name="ydrain", bufs=4) as y_pool,
            tc.tile_pool(name="warm", bufs=1) as warm_pool,
        ):
            # ---- HAM warm-up: PE busy from program start (no DMA deps) ----
            wu_sb = warm_pool.tile([P, 512], bf16)
            nc.gpsimd.memset(wu_sb, 0.0)

            # ---- head DMAs, priority order, spread over both HWDGE rings ----
            # sync ring: weights; scalar ring: xg (w2 prefetch on gpsimd).
            # The first matmul's deps are split out tiny (w1 d0 slice + xg
            # d0/cc0 chunk) so it can fire ~2us earlier.
            w1h0c_r = w1h0c[:].rearrange("p (do h) -> p do h", do=KD)
            w3h0c_r = w3h0c[:].rearrange("p (do h) -> p do h", do=KD)
            w1_sb0 = w13_pool.tile([P, KD, P], bf16, tag="w1h0")
            nc.sync.dma_start(out=w1_sb0[:, 0:1, :], in_=w1h0c_r[:, 0:1, :])
            xg_sb = xg_pool.tile([P, KD, C], bf16)
            c00, cw0 = CCH[0]
            nc.scalar.dma_start(
                out=xg_sb[:, 0, c00 : c00 + cw0], in_=xgt_r[:, 0, c00 : c00 + cw0]
            )
            nc.sync.dma_start(out=w1_sb0[:, 1:KD, :], in_=w1h0c_r[:, 1:KD, :])
            for c0, cw in CCH[1:]:
                nc.scalar.dma_start(
                    out=xg_sb[:, 0, c0 : c0 + cw], in_=xgt_r[:, 0, c0 : c0 + cw]
                )
            w3_sb0 = w13_pool.tile([P, KD, P], bf16, tag="w3h0")
            nc.sync.dma_start(out=w3_sb0, in_=w3h0c_r)
            # xg d1..d7: split across both rings so the head window uses
            # the full DMA bandwidth for the critical stream
            for d in range(1, KD):
                eng = nc.sync if d in (1, 3) else nc.scalar
                eng.dma_start(out=xg_sb[:, d, :], in_=xgt_r[:, d, :])
            # preload the Silu activation table on the scalar engine (after
            # the head DMA issues: the ~1.3us table load must not delay
            # them, but must finish before the first real silu drain)
            silu_warm = scratch_pool.tile([P, 8], f32, tag="actwarm")
            nc.scalar.activation(silu_warm, wu_sb[:, 0:8], SILU)

            a_lo = a_pool.tile([P, NHL, C], bf16, tag="alo")
            a_hi = a_pool.tile([P, NH - NHL, C], bf16, tag="ahi")

            def a_slice(h, c0, cw):
                if h < NHL:
                    return a_lo[:, h, c0 : c0 + cw]
                return a_hi[:, h - NHL, c0 : c0 + cw]

            w2_sb = w2_pool.tile([P, NH, Dm], bf16)

            # ---- one PSUM pool for warmup + both phases: no pool-close
            # ---- barrier between phase 1's last drains and phase 2
            with tc.tile_pool(name="ps", bufs=8, space="PSUM") as ps1:
                ps2 = ps1
                wu_ps = ps1.tile([P, 512], f32, tag="ps", name="warm")
                for _ in range(3):
                    nc.tensor.matmul(wu_ps, wu_sb[:, 0:P], wu_sb, start=True, stop=True)

                # ---- phase 1: a[h, c] = silu(w1.T x) * (w3.T x) ----
                for h in range(NH):
                    hs = slice(h * P, (h + 1) * P)
                    if h == 0:
                        w1_sb, w3_sb = w1_sb0, w3_sb0
                    else:
                        w1_sb = w13_pool.tile([P, KD, P], bf16, tag="w1")
                        nc.sync.dma_start(out=w1_sb, in_=w1t_r[:, :, hs])
                        w3_sb = w13_pool.tile([P, KD, P], bf16, tag="w3")
                        nc.sync.dma_start(out=w3_sb, in_=w3t_r[:, :, hs])
                        # w2 prefetch for phase 2: scalar ring, starting only
                        # after the head xg/w13 window (the first ~25us of
                        # DMA is saturated with critical-path traffic)
                        if h >= 6:
                            for h2 in (2 * (h - 6), 2 * (h - 6) + 1):
                                if h2 < NH:
                                    nc.scalar.dma_start(
                                        out=w2_sb[:, h2, :],
                                        in_=w2t[h2 * P : (h2 + 1) * P, :],
                                    )

                    h1_ps = [
                        ps1.tile([P, cw], f32, tag="ps", name=f"h1_{h}_{i}")
                        for i, (c0, cw) in enumerate(CCH)
                    ]
                    h3_ps = [
                        ps1.tile([P, cw], f32, tag="ps", name=f"h3_{h}_{i}")
                        for i, (c0, cw) in enumerate(CCH)
                    ]
                    # h==0 is paced by xg arrival: interleave w1/w3 per
                    # d-tile so each arriving xg slice unlocks 4 matmuls.
                    if h == 0:
                        wave = [(w1_sb, h1_ps), (w3_sb, h3_ps)]
                        for d in range(KD):
                            for w_sb, ps in wave:
                                for i, (c0, cw) in enumerate(CCH):
                                    nc.tensor.matmul(
                                        ps[i],
                                        w_sb[:, d, :],
                                        xg_sb[:, d, c0 : c0 + cw],
                                        start=(d == 0),
                                        stop=(d == KD - 1),
                                    )
                    else:
                        for d in range(KD):
                            for i, (c0, cw) in enumerate(CCH):
                                nc.tensor.matmul(
                                    h1_ps[i],
                                    w1_sb[:, d, :],
                                    xg_sb[:, d, c0 : c0 + cw],
                                    start=(d == 0),
                                    stop=(d == KD - 1),
                                )
                        for d in range(KD):
                            for i, (c0, cw) in enumerate(CCH):
                                nc.tensor.matmul(
                                    h3_ps[i],
                                    w3_sb[:, d, :],
                                    xg_sb[:, d, c0 : c0 + cw],
                                    start=(d == 0),
                                    stop=(d == KD - 1),
                                )
                    for i, (c0, cw) in enumerate(CCH):
                        s_sb = scratch_pool.tile([P, 512], f32, tag="scratch")
                        nc.scalar.activation(s_sb[:, 0:cw], h1_ps[i], SILU)
                        nc.vector.tensor_mul(
                            out=a_slice(h, c0, cw),
                            in0=s_sb[:, 0:cw],
                            in1=h3_ps[i],
                        )

                # ---- phase 2: yT[dd, c] = sum_h w2T[h, dd].T @ a[h, c] ----
                # dd-groups sized [2,2,2,1,1]: drains overlap the next
                # group's matmuls and the final drain is small.
                ddgs = []
                nd = 0
                for g in (2, 2, 2, 1, 1):
                    ddgs.append(list(range(nd, min(nd + g, ND))))
                    nd += g
                assert nd == ND, (nd, ND)

                for gi, dds in enumerate(ddgs):
                    y_ps = {
                        (dd, i): ps2.tile([P, cw], f32, tag="ps", name=f"y_{dd}_{i}")
                        for dd in dds
                        for i, (c0, cw) in enumerate(CCH)
                    }
                    for h in range(NH):
                        for dd in dds:
                            w2_w = w2_sb[:, h, dd * P : (dd + 1) * P]
                            for i, (c0, cw) in enumerate(CCH):
                                nc.tensor.matmul(
                                    y_ps[(dd, i)],
                                    w2_w,
                                    a_slice(h, c0, cw),
                                    start=(h == 0),
                                    stop=(h == NH - 1),
                                )
                    last = gi == len(ddgs) - 1
                    for dd in dds:
                        for i, (c0, cw) in enumerate(CCH):
                            y_sb = y_pool.tile(
                                [P, 512], bf16, tag="y", name=f"ysb_{dd}_{i}"
                            )
                            if last and i % 2 == 1:
                                # parallelize the final drain across engines
                                nc.scalar.activation(
                                    y_sb[:, 0:cw], y_ps[(dd, i)], COPY
                                )
                                nc.scalar.dma_start(
                                    out=yt[dd * P : (dd + 1) * P, c0 : c0 + cw],
                                    in_=y_sb[:, 0:cw],
                                )
                            else:
                                nc.vector.tensor_copy(
                                    out=y_sb[:, 0:cw], in_=y_ps[(dd, i)]
                                )
                                nc.sync.dma_start(
                                    out=yt[dd * P : (dd + 1) * P, c0 : c0 + cw],
                                    in_=y_sb[:, 0:cw],
                                )
    nc.compile()  # bacc passes: split multi-waits, alloc regs, fuse nops
    return nc


def _get_program(C: int, Dm: int, Hm: int):
    key = (C, Dm, Hm)
    if key not in _PROG_CACHE:
        _PROG_CACHE[key] = _build_program(C, Dm, Hm)
    return _PROG_CACHE[key]


def kernel(x, expert_indices, w1, w2, w3):
    global LAST_RUN
    from concourse.bass_utils import run_bass_kernel_spmd

    x = np.ascontiguousarray(np.asarray(x, dtype=np.float32))
    idx = np.asarray(expert_indices)
    w1 = np.asarray(w1, dtype=np.float32)
    w2 = np.asarray(w2, dtype=np.float32)
    w3 = np.asarray(w3, dtype=np.float32)

    Tn, Kn = idx.shape
    Dm = x.shape[1]
    En, Hm, _ = w1.shape
    assert En == 8, f"kernel is hardcoded for 8 experts on 8 cores, got {En}"
    idx64 = idx.astype(np.int64)
    KD = Dm // P

    # Host routing: unique token list per expert.
    toks = [np.nonzero((idx64 == e).any(axis=1))[0] for e in range(En)]
    maxc = max(len(t) for t in toks)
    C = max(512, -(-maxc // 8) * 8)

    nc = _get_program(C, Dm, Hm)

    in_maps = []
    for e in range(En):
        te = toks[e]
        xg = np.zeros((C, Dm), np.float32)
        xg[: len(te)] = x[te]
        w1te = np.ascontiguousarray(w1[e].T.astype(BF16))  # [Dm, Hm]
        w3te = np.ascontiguousarray(w3[e].T.astype(BF16))
        # first h-tile pre-tiled: [di, do, hh] contiguous per partition
        w1h0c = np.ascontiguousarray(
            w1te[:, 0:P].reshape(KD, P, P).transpose(1, 0, 2).reshape(P, KD * P)
        )
        w3h0c = np.ascontiguousarray(
            w3te[:, 0:P].reshape(KD, P, P).transpose(1, 0, 2).reshape(P, KD * P)
        )
        in_maps.append(
            {
                "xgt": np.ascontiguousarray(xg.T.astype(BF16)),
                "w1t": w1te,
                "w3t": w3te,
                "w2t": np.ascontiguousarray(w2[e].T.astype(BF16)),
                "w1h0c": w1h0c,
                "w3h0c": w3h0c,
            }
        )

    LAST_RUN = run_bass_kernel_spmd(nc, in_maps, list(range(En)))
    res = LAST_RUN.results

    out = np.empty((Tn, Kn, Dm), np.float32)
    for e in range(En):
        ye = np.asarray(res[e]["yt"]).astype(np.float32).T  # [C, Dm]
        t_arr, k_arr = np.nonzero(idx64 == e)
        pos = np.searchsorted(toks[e], t_arr)
        out[t_arr, k_arr] = ye[pos]
    return out
